# revision 13
# baseline (speedup 1.0000x reference)
"""ClusterScaleBiasBlock Trainium2 kernel.

Computes out = BN(x) * (1 + Wg[ids]) + Wb[ids] for
x:[32768,2048] f32, Wg/Wb:[64,2048], ids:[32768] int32, where
BN(x) = (x - mean) * rsqrt(var+eps) * gamma + beta (inference mode).

Algebraic folding (host side, tiny [64,2048] tables):
    inv  = rsqrt(var + eps) * gamma
    S[c] = inv * (1 + Wg[c])
    T[c] = (beta - mean*inv) * (1 + Wg[c]) + Wb[c]
    out  = x * S[ids] + T[ids]

Layout strategy (the kernel is HBM-bandwidth-bound, so minimize traffic):
  - Shard by CLUSTER, not by batch row: core c owns 8 whole clusters, so
    every row a core touches uses one of 8 (scale, bias) vector pairs.
  - Clusters are rank-matched into 8 "slots" (slot j = clusters with
    size-rank 8j..8j+7, one per core) and each slot is padded to the max
    size in its octile -> all 8 cores share identical slot extents, so a
    single SPMD program works; padding is only ~1-2%.
  - Host transposes x rows into feature-major [2048, R] bf16 tiles.  With
    features on partitions, out = x*s + t needs just ONE VectorE
    tensor_scalar instruction per (feature-tile, slot): s,t are
    per-partition scalar APs.  bf16 + unit stride -> DVE 4x mode.
  - x/out move as bf16 (tolerance is 2e-2; bf16 gives ~4e-3), halving
    HBM traffic vs f32: ~34 MB/core total vs 64 MB for the f32 baseline.
  - Loads ride the SP HWDGE ring, stores the ACT ring.
"""

import sys

if "/opt/trn_rl_repo" not in sys.path:
    sys.path.insert(0, "/opt/trn_rl_repo")

import numpy as np

B, F, C = 32768, 2048, 64
N_CORES = 8
P = 128
NFT = F // P          # 16 feature tiles of 128 partitions
BN_EPS = 1e-3

_PROGRAM = None
_PROG_KEY = None


def _build_program(R, M):
    """R = padded rows per core; M = per-slot column extents (sum == R)."""
    import concourse.bacc as bacc
    import concourse.mybir as mybir
    from concourse import tile

    f32 = mybir.dt.float32
    bf16 = mybir.dt.bfloat16
    nslots = len(M)
    q = [0]
    for m in M:
        q.append(q[-1] + m)

    nc = bacc.Bacc(None)
    x_d = nc.declare_dram_parameter("xt", [F, R], bf16, isOutput=False)
    s_d = nc.declare_dram_parameter("stab", [P, NFT * nslots], f32, isOutput=False)
    t_d = nc.declare_dram_parameter("ttab", [P, NFT * nslots], f32, isOutput=False)
    o_d = nc.declare_dram_parameter("ot", [F, R], bf16, isOutput=True)

    mult = mybir.AluOpType.mult
    add = mybir.AluOpType.add

    # Column chunking keeps SBUF tiles bounded for any cluster-size
    # distribution; with uniform ids R ~= 4141 and there is one chunk.
    CH = 4608
    nck = (R + CH - 1) // CH
    cuts = [min(R, k * CH) for k in range(nck + 1)]
    tspecs = [(ft, cuts[k], cuts[k + 1])
              for ft in range(NFT) for k in range(nck)]
    nt = len(tspecs)
    wmax = max(c1 - c0 for _, c0, c1 in tspecs)

    # Each HWDGE ring (SP=sync, ACT=scalar) carries half the loads AND
    # half the stores, alternating by tile, so both rings stream from the
    # first tile to the last (a dedicated store ring would idle for the
    # first ~12us and a dedicated load ring for the last ~10us).
    # Transfers stay full-tile (~1 MB, 8 KB per partition line) -- halving
    # them measurably drops per-ring throughput.  Stores are emitted two
    # tiles behind loads so neither ring head-of-line blocks on compute.
    with tile.TileContext(nc) as tc:
        with (
            tc.tile_pool(name="const", bufs=1) as cpool,
            tc.tile_pool(name="xin", bufs=5) as xpool,
            tc.tile_pool(name="oout", bufs=5) as opool,
        ):
            s_sb = cpool.tile([P, NFT * nslots], f32, tag="stab")
            t_sb = cpool.tile([P, NFT * nslots], f32, tag="ttab")
            WSCR = 8960  # one ~2.8us pacing op per tile (620ns + 0.245ns/col)
            scr_a = cpool.tile([P, WSCR], bf16, tag="scr_a")
            scr_b = cpool.tile([P, WSCR], bf16, tag="scr_b")
            nc.sync.dma_start(out=s_sb[:], in_=s_d[:])
            nc.scalar.dma_start(out=t_sb[:], in_=t_d[:])
            nc.vector.memset(scr_a[:], 0.0)
            nc.vector.memset(scr_b[:], 0.0)

            xts = {}

            def emit_load(i):
                ft, c0, c1 = tspecs[i]
                ld = nc.sync if i % 2 == 0 else nc.scalar
                rows = slice(ft * P, (ft + 1) * P)
                w = c1 - c0
                xt = xpool.tile([P, w], bf16, tag="x")
                if i < 2:
                    # split the first load on each ring: compute starts
                    # after half a tile is in
                    h = (w // 4) * 2
                    ld.dma_start(out=xt[:, 0:h], in_=x_d[rows, c0:c0 + h])
                    ld.dma_start(out=xt[:, h:], in_=x_d[rows, c0 + h:c1])
                else:
                    ld.dma_start(out=xt[:], in_=x_d[rows, c0:c1])
                xts[i] = xt

            def emit_compute_store(i):
                ft, c0, c1 = tspecs[i]
                st = nc.scalar if i % 2 == 0 else nc.sync
                rows = slice(ft * P, (ft + 1) * P)
                w = c1 - c0
                xt = xts.pop(i)
                ot = opool.tile([P, w], bf16, tag="o")
                for j in range(nslots):
                    a = max(q[j], c0)
                    b = min(q[j + 1], c1)
                    if a >= b:
                        continue
                    cs = slice(a - c0, b - c0)
                    col = ft * nslots + j
                    nc.vector.tensor_scalar(
                        ot[:, cs], xt[:, cs],
                        s_sb[:, col:col + 1], t_sb[:, col:col + 1],
                        mult, add)
                if i >= nt - 2:
                    # split the last store on each ring: the drain is
                    # latency-bound once loads have finished
                    h = (w // 4) * 2
                    st.dma_start(out=o_d[rows, c0:c0 + h], in_=ot[:, 0:h])
                    st.dma_start(out=o_d[rows, c0 + h:c1], in_=ot[:, h:])
                else:
                    st.dma_start(out=o_d[rows, c0:c1], in_=ot[:])
                # Pacing: the two NeuronCores sharing an HBM stack issue
                # identical demand, but arbitration favors one of them
                # (~400 vs ~330 GB/s observed), and the score is the
                # slowest core.  Dummy VectorE ops stretch the DVE stream
                # so the next tile's compute -- and hence the x-pool
                # recycle that gates the next load -- can't run ahead of
                # the fair-share rate (~90us/16 tiles).  They read/write
                # only scratch: on a core whose DMA is already slow the
                # dummies complete while it waits for data, so pacing
                # binds the arbitration "winner" without throttling the
                # "loser".  Skipped on the first/last tiles so the head
                # fills and the tail drains greedily.
                if 1 <= i <= nt - 3:
                    src, dst = (scr_a, scr_b) if i % 2 else (scr_b, scr_a)
                    nc.vector.tensor_scalar(
                        dst[:], src[:],
                        s_sb[:, 0:1], t_sb[:, 0:1], mult, add)

            for i in range(nt + 2):
                if i < nt:
                    emit_load(i)
                if i >= 2:
                    emit_compute_store(i - 2)
    nc.compile()
    return nc


def _host_tables(Wg, Wb, bn_gamma, bn_beta, moving_mean, moving_var):
    inv = (bn_gamma.astype(np.float64)
           / np.sqrt(moving_var.astype(np.float64) + BN_EPS))
    gp1 = 1.0 + Wg.astype(np.float64)  # [C, F]
    S = (inv[None, :] * gp1).astype(np.float32)
    T = ((bn_beta.astype(np.float64) - moving_mean.astype(np.float64) * inv)[None, :]
         * gp1 + Wb.astype(np.float64)).astype(np.float32)
    return S, T


def kernel(x, Wg, Wb, bn_gamma, bn_beta, moving_mean, moving_var, cluster_ids):
    global _PROGRAM, _PROG_KEY
    import ml_dtypes
    from concourse.bass_utils import run_bass_kernel_spmd

    bf16 = ml_dtypes.bfloat16
    x = np.asarray(x, dtype=np.float32)
    ids = np.asarray(cluster_ids, dtype=np.int32)
    S, T = _host_tables(
        np.asarray(Wg, np.float32), np.asarray(Wb, np.float32),
        np.asarray(bn_gamma, np.float32), np.asarray(bn_beta, np.float32),
        np.asarray(moving_mean, np.float32), np.asarray(moving_var, np.float32),
    )

    counts = np.bincount(ids, minlength=C)
    present = np.nonzero(counts)[0]
    ranked = present[np.argsort(-counts[present], kind="stable")]
    npad = (-len(ranked)) % N_CORES
    ranked = np.concatenate([ranked, np.full(npad, -1, dtype=np.int64)])
    nslots = len(ranked) // N_CORES
    slot_cl = ranked.reshape(nslots, N_CORES)   # [slot, core] -> cluster id
    # slot extents: max cluster size in the octile, rounded up to even
    M = []
    for j in range(nslots):
        mx = max((int(counts[cl]) for cl in slot_cl[j] if cl >= 0), default=0)
        M.append(max(2, ((mx + 1) // 2) * 2))
    R = int(sum(M))
    q = [0]
    for m in M:
        q.append(q[-1] + m)

    order = np.argsort(ids, kind="stable")
    starts = np.zeros(C + 1, dtype=np.int64)
    np.cumsum(counts, out=starts[1:])

    in_maps = []
    idx_all, cnt_all = [], []
    for c in range(N_CORES):
        idx = np.empty(R, dtype=np.int64)
        cnts = np.zeros(nslots, dtype=np.int64)
        stab = np.zeros((F, nslots), dtype=np.float32)
        ttab = np.zeros((F, nslots), dtype=np.float32)
        for j in range(nslots):
            a, b = q[j], q[j + 1]
            cl = int(slot_cl[j, c])
            if cl < 0:
                idx[a:b] = order[0]     # scale/bias stay 0; never scattered
                continue
            n = int(counts[cl])
            rows = order[starts[cl]:starts[cl] + n]
            idx[a:a + n] = rows
            idx[a + n:b] = rows[0]      # pad with a row of the same cluster
            cnts[j] = n
            stab[:, j] = S[cl]
            ttab[:, j] = T[cl]
        idx_all.append(idx)
        cnt_all.append(cnts)
        # feature-major bf16 transpose of this core's rows: [F, R]
        xt = x[idx].T.astype(bf16, order="C")
        st = stab.reshape(NFT, P, nslots).transpose(1, 0, 2).reshape(P, NFT * nslots)
        tt = ttab.reshape(NFT, P, nslots).transpose(1, 0, 2).reshape(P, NFT * nslots)
        in_maps.append({
            "xt": xt,
            "stab": np.ascontiguousarray(st),
            "ttab": np.ascontiguousarray(tt),
        })

    key = (R, tuple(M))
    if _PROGRAM is None or _PROG_KEY != key:
        _PROGRAM = _build_program(R, M)
        _PROG_KEY = key

    res = run_bass_kernel_spmd(_PROGRAM, in_maps, list(range(N_CORES)))
    globals()["LAST_RESULT"] = res

    out = np.empty((B, F), dtype=np.float32)
    for c in range(N_CORES):
        z = np.asarray(res.results[c]["ot"])          # [F, R] bf16
        zf = z.T.astype(np.float32, order="C")        # [R, F]
        idx, cnts = idx_all[c], cnt_all[c]
        for j in range(nslots):
            a, n = q[j], int(cnts[j])
            if n:
                out[idx[a:a + n]] = zf[a:a + n]
    return out


if __name__ == "__main__":
    # Smoke test with random data against a local numpy reference.
    rng = np.random.default_rng(0)
    inputs = {
        "x": rng.standard_normal((B, F), dtype=np.float32),
        "Wg": 0.25 * rng.standard_normal((C, F)).astype(np.float32),
        "Wb": 0.25 * rng.standard_normal((C, F)).astype(np.float32),
        "bn_gamma": np.ones(F, np.float32),
        "bn_beta": np.zeros(F, np.float32),
        "moving_mean": 0.1 * rng.standard_normal(F).astype(np.float32),
        "moving_var": rng.uniform(0.5, 1.5, F).astype(np.float32),
        "cluster_ids": rng.integers(0, C, B, dtype=np.int32),
    }
    out = kernel(**inputs)
    inv = inputs["bn_gamma"] / np.sqrt(inputs["moving_var"] + BN_EPS)
    xn = (inputs["x"] - inputs["moving_mean"]) * inv + inputs["bn_beta"]
    g = inputs["Wg"][inputs["cluster_ids"]]
    b = inputs["Wb"][inputs["cluster_ids"]]
    ref = xn * (1.0 + g) + b
    err = np.max(np.abs(out - ref)) / np.max(np.abs(ref))
    print("rel err:", err)


# revision 16
# speedup vs baseline: 1.1988x; 1.1988x over previous
"""ClusterScaleBiasBlock Trainium2 kernel.

Computes out = BN(x) * (1 + Wg[ids]) + Wb[ids] for
x:[32768,2048] f32, Wg/Wb:[64,2048], ids:[32768] int32, where
BN(x) = (x - mean) * rsqrt(var+eps) * gamma + beta (inference mode).

Algebraic folding (host side, tiny [64,2048] tables):
    inv  = rsqrt(var + eps) * gamma
    S[c] = inv * (1 + Wg[c])
    T[c] = (beta - mean*inv) * (1 + Wg[c]) + Wb[c]
    out  = x * S[ids] + T[ids]

Layout strategy (the kernel is HBM-bandwidth-bound, so minimize traffic):
  - Shard by CLUSTER, not by batch row: core c owns 8 whole clusters, so
    every row a core touches uses one of 8 (scale, bias) vector pairs.
  - Clusters are rank-matched into 8 "slots" (slot j = clusters with
    size-rank 8j..8j+7, one per core) and each slot is padded to the max
    size in its octile -> all 8 cores share identical slot extents, so a
    single SPMD program works; padding is only ~1-2%.
  - Host transposes x rows into feature-major [2048, R] bf16 tiles.  With
    features on partitions, out = x*s + t needs just ONE VectorE
    tensor_scalar instruction per (feature-tile, slot): s,t are
    per-partition scalar APs.  bf16 + unit stride -> DVE 4x mode.
  - x/out move as bf16 (tolerance is 2e-2; bf16 gives ~4e-3), halving
    HBM traffic vs f32: ~34 MB/core total vs 64 MB for the f32 baseline.
  - Loads ride the SP HWDGE ring, stores the ACT ring.
"""

import sys

if "/opt/trn_rl_repo" not in sys.path:
    sys.path.insert(0, "/opt/trn_rl_repo")

import numpy as np

B, F, C = 32768, 2048, 64
N_CORES = 8
P = 128
NFT = F // P          # 16 feature tiles of 128 partitions
BN_EPS = 1e-3

_PROGRAM = None
_PROG_KEY = None


def _build_program(R, M):
    """R = padded rows per core; M = per-slot column extents (sum == R)."""
    import concourse.bacc as bacc
    import concourse.mybir as mybir
    from concourse import tile

    f32 = mybir.dt.float32
    bf16 = mybir.dt.bfloat16
    nslots = len(M)
    q = [0]
    for m in M:
        q.append(q[-1] + m)

    nc = bacc.Bacc(None)
    x_d = nc.declare_dram_parameter("xt", [F, R], bf16, isOutput=False)
    s_d = nc.declare_dram_parameter("stab", [P, NFT * nslots], f32, isOutput=False)
    t_d = nc.declare_dram_parameter("ttab", [P, NFT * nslots], f32, isOutput=False)
    o_d = nc.declare_dram_parameter("ot", [F, R], bf16, isOutput=True)

    mult = mybir.AluOpType.mult
    add = mybir.AluOpType.add

    # Column chunking keeps SBUF tiles bounded for any cluster-size
    # distribution; with uniform ids R ~= 4141 and there is one chunk.
    CH = 4608
    nck = (R + CH - 1) // CH
    cuts = [min(R, k * CH) for k in range(nck + 1)]
    tspecs = [(ft, cuts[k], cuts[k + 1])
              for ft in range(NFT) for k in range(nck)]
    nt = len(tspecs)
    wmax = max(c1 - c0 for _, c0, c1 in tspecs)

    # Each HWDGE ring (SP=sync, ACT=scalar) carries half the loads AND
    # half the stores, alternating by tile, so both rings stream from the
    # first tile to the last (a dedicated store ring would idle for the
    # first ~12us and a dedicated load ring for the last ~10us).
    # Transfers stay full-tile (~1 MB, 8 KB per partition line) -- halving
    # them measurably drops per-ring throughput.  Stores are emitted two
    # tiles behind loads so neither ring head-of-line blocks on compute.
    with tile.TileContext(nc) as tc:
        with (
            tc.tile_pool(name="const", bufs=1) as cpool,
            tc.tile_pool(name="xin", bufs=5) as xpool,
            tc.tile_pool(name="oout", bufs=5) as opool,
        ):
            s_sb = cpool.tile([P, NFT * nslots], f32, tag="stab")
            t_sb = cpool.tile([P, NFT * nslots], f32, tag="ttab")
            nc.sync.dma_start(out=s_sb[:], in_=s_d[:])
            nc.scalar.dma_start(out=t_sb[:], in_=t_d[:])

            xts = {}

            def emit_load(i):
                ft, c0, c1 = tspecs[i]
                ld = nc.sync if i % 2 == 0 else nc.scalar
                rows = slice(ft * P, (ft + 1) * P)
                w = c1 - c0
                xt = xpool.tile([P, w], bf16, tag="x")
                if i < 2:
                    # split the first load on each ring: compute starts
                    # after half a tile is in
                    h = (w // 4) * 2
                    ld.dma_start(out=xt[:, 0:h], in_=x_d[rows, c0:c0 + h])
                    ld.dma_start(out=xt[:, h:], in_=x_d[rows, c0 + h:c1])
                else:
                    ld.dma_start(out=xt[:], in_=x_d[rows, c0:c1])
                xts[i] = xt

            def emit_compute_store(i):
                ft, c0, c1 = tspecs[i]
                # stores ride the SWDGE queue (own engine, own descriptor
                # stream): a store whose compute isn't done yet must never
                # sit in front of a load in an HWDGE ring's FIFO -- that
                # head-of-line stall idles the SDMA engines on any core
                # whose data is arriving late, compounding its lag.
                st = nc.gpsimd
                rows = slice(ft * P, (ft + 1) * P)
                w = c1 - c0
                xt = xts.pop(i)
                ot = opool.tile([P, w], bf16, tag="o")
                for j in range(nslots):
                    a = max(q[j], c0)
                    b = min(q[j + 1], c1)
                    if a >= b:
                        continue
                    cs = slice(a - c0, b - c0)
                    col = ft * nslots + j
                    nc.vector.tensor_scalar(
                        ot[:, cs], xt[:, cs],
                        s_sb[:, col:col + 1], t_sb[:, col:col + 1],
                        mult, add)
                if i >= nt - 2:
                    # split the last store on each ring: the drain is
                    # latency-bound once loads have finished
                    h = (w // 4) * 2
                    st.dma_start(out=o_d[rows, c0:c0 + h], in_=ot[:, 0:h])
                    st.dma_start(out=o_d[rows, c0 + h:c1], in_=ot[:, h:])
                else:
                    st.dma_start(out=o_d[rows, c0:c1], in_=ot[:])
            for i in range(nt + 2):
                if i < nt:
                    emit_load(i)
                if i >= 2:
                    emit_compute_store(i - 2)
    nc.compile()
    return nc


def _host_tables(Wg, Wb, bn_gamma, bn_beta, moving_mean, moving_var):
    inv = (bn_gamma.astype(np.float64)
           / np.sqrt(moving_var.astype(np.float64) + BN_EPS))
    gp1 = 1.0 + Wg.astype(np.float64)  # [C, F]
    S = (inv[None, :] * gp1).astype(np.float32)
    T = ((bn_beta.astype(np.float64) - moving_mean.astype(np.float64) * inv)[None, :]
         * gp1 + Wb.astype(np.float64)).astype(np.float32)
    return S, T


def kernel(x, Wg, Wb, bn_gamma, bn_beta, moving_mean, moving_var, cluster_ids):
    global _PROGRAM, _PROG_KEY
    import ml_dtypes
    from concourse.bass_utils import run_bass_kernel_spmd

    bf16 = ml_dtypes.bfloat16
    x = np.asarray(x, dtype=np.float32)
    ids = np.asarray(cluster_ids, dtype=np.int32)
    S, T = _host_tables(
        np.asarray(Wg, np.float32), np.asarray(Wb, np.float32),
        np.asarray(bn_gamma, np.float32), np.asarray(bn_beta, np.float32),
        np.asarray(moving_mean, np.float32), np.asarray(moving_var, np.float32),
    )

    counts = np.bincount(ids, minlength=C)
    present = np.nonzero(counts)[0]
    ranked = present[np.argsort(-counts[present], kind="stable")]
    npad = (-len(ranked)) % N_CORES
    ranked = np.concatenate([ranked, np.full(npad, -1, dtype=np.int64)])
    nslots = len(ranked) // N_CORES
    slot_cl = ranked.reshape(nslots, N_CORES)   # [slot, core] -> cluster id
    # slot extents: max cluster size in the octile, rounded up to even
    M = []
    for j in range(nslots):
        mx = max((int(counts[cl]) for cl in slot_cl[j] if cl >= 0), default=0)
        M.append(max(2, ((mx + 1) // 2) * 2))
    R = int(sum(M))
    q = [0]
    for m in M:
        q.append(q[-1] + m)

    order = np.argsort(ids, kind="stable")
    starts = np.zeros(C + 1, dtype=np.int64)
    np.cumsum(counts, out=starts[1:])

    in_maps = []
    idx_all, cnt_all = [], []
    for c in range(N_CORES):
        idx = np.empty(R, dtype=np.int64)
        cnts = np.zeros(nslots, dtype=np.int64)
        stab = np.zeros((F, nslots), dtype=np.float32)
        ttab = np.zeros((F, nslots), dtype=np.float32)
        for j in range(nslots):
            a, b = q[j], q[j + 1]
            cl = int(slot_cl[j, c])
            if cl < 0:
                idx[a:b] = order[0]     # scale/bias stay 0; never scattered
                continue
            n = int(counts[cl])
            rows = order[starts[cl]:starts[cl] + n]
            idx[a:a + n] = rows
            idx[a + n:b] = rows[0]      # pad with a row of the same cluster
            cnts[j] = n
            stab[:, j] = S[cl]
            ttab[:, j] = T[cl]
        idx_all.append(idx)
        cnt_all.append(cnts)
        # feature-major bf16 transpose of this core's rows: [F, R]
        xt = x[idx].T.astype(bf16, order="C")
        st = stab.reshape(NFT, P, nslots).transpose(1, 0, 2).reshape(P, NFT * nslots)
        tt = ttab.reshape(NFT, P, nslots).transpose(1, 0, 2).reshape(P, NFT * nslots)
        in_maps.append({
            "xt": xt,
            "stab": np.ascontiguousarray(st),
            "ttab": np.ascontiguousarray(tt),
        })

    key = (R, tuple(M))
    if _PROGRAM is None or _PROG_KEY != key:
        _PROGRAM = _build_program(R, M)
        _PROG_KEY = key

    res = run_bass_kernel_spmd(_PROGRAM, in_maps, list(range(N_CORES)))
    globals()["LAST_RESULT"] = res

    out = np.empty((B, F), dtype=np.float32)
    for c in range(N_CORES):
        z = np.asarray(res.results[c]["ot"])          # [F, R] bf16
        zf = z.T.astype(np.float32, order="C")        # [R, F]
        idx, cnts = idx_all[c], cnt_all[c]
        for j in range(nslots):
            a, n = q[j], int(cnts[j])
            if n:
                out[idx[a:a + n]] = zf[a:a + n]
    return out


if __name__ == "__main__":
    # Smoke test with random data against a local numpy reference.
    rng = np.random.default_rng(0)
    inputs = {
        "x": rng.standard_normal((B, F), dtype=np.float32),
        "Wg": 0.25 * rng.standard_normal((C, F)).astype(np.float32),
        "Wb": 0.25 * rng.standard_normal((C, F)).astype(np.float32),
        "bn_gamma": np.ones(F, np.float32),
        "bn_beta": np.zeros(F, np.float32),
        "moving_mean": 0.1 * rng.standard_normal(F).astype(np.float32),
        "moving_var": rng.uniform(0.5, 1.5, F).astype(np.float32),
        "cluster_ids": rng.integers(0, C, B, dtype=np.int32),
    }
    out = kernel(**inputs)
    inv = inputs["bn_gamma"] / np.sqrt(inputs["moving_var"] + BN_EPS)
    xn = (inputs["x"] - inputs["moving_mean"]) * inv + inputs["bn_beta"]
    g = inputs["Wg"][inputs["cluster_ids"]]
    b = inputs["Wb"][inputs["cluster_ids"]]
    ref = xn * (1.0 + g) + b
    err = np.max(np.abs(out - ref)) / np.max(np.abs(ref))
    print("rel err:", err)
